# revision 5
# baseline (speedup 1.0000x reference)
"""Trainium2 Bass kernel for EnhancedLegalRGCN (3-layer RGCN + edge/node heads).

Strategy (8 NeuronCores, SPMD):
- Nodes block-sharded 8 ways (6250/core). Edges owned by dst core, packed into
  dst-window-aligned slots (49 windows x T_W tiles x 128 edges, padded).
- Per layer: sharded transform GEMM (x^T tiles @ [W_rel|W_root]) -> AllGather
  bf16 table [50000, 384] in 5 chunks -> per-edge indirect DMA gather of
  (src, rel) message rows -> scatter via host-built one-hot*1/cnt S matrices
  with TensorE matmuls accumulating each dst window in PSUM -> +root, ReLU.
- Edge head: u = x3@We1[:H] gathered per edge; v = x3@We1[H:] expanded from the
  local dst window by a one-hot matmul; hidden^T built directly in PSUM
  (transpose-matmul + v-expansion matmul), ReLU+bias on ScalarE, @We2.
- Node head: local GEMMs in transposed space.
Host reassembles/unpermutes outputs.
"""
import numpy as np
import ml_dtypes

BF16 = ml_dtypes.bfloat16
LAST_EXEC_NS = None

N, E, IN, H, R = 50000, 800000, 768, 128, 3
NC = 8
SHARD = N // NC          # 6250
NWIN = 49                # ceil(6250/128)
NAG = 5
CHUNK = SHARD // NAG     # 1250
NPAD = NWIN * 128        # 6272


def _table_row(n):
    k = n // SHARD
    m = n % SHARD
    c = m // CHUNK
    p = m % CHUNK
    return c * (NC * CHUNK) + k * CHUNK + p


def _build_plan(src, dst, et):
    cnt = np.zeros((R, N), dtype=np.int64)
    np.add.at(cnt, (et, dst), 1)
    scale_rn = 1.0 / np.maximum(cnt, 1).astype(np.float32)
    s_e = scale_rn[et, dst].astype(np.float32)

    core = dst // SHARD
    dst_local = dst % SHARD
    win = dst_local // 128

    order = np.lexsort((win, core))
    cw_cnt = np.zeros((NC, NWIN), dtype=np.int64)
    np.add.at(cw_cnt, (core, win), 1)
    T_W = int(np.ceil(cw_cnt.max() / 128))
    nt = NWIN * T_W
    n_slots = nt * 128

    tr = _table_row(np.arange(N))

    class P:
        pass
    plan = P()
    plan.T_W = T_W
    plan.NT = nt
    plan.msg_idx = np.zeros((NC, 128, nt), dtype=np.int32)
    plan.u_idx = np.zeros((NC, 128, nt), dtype=np.int32)
    plan.S = np.zeros((NC, 128, nt * 128), dtype=BF16)
    plan.S0T = np.zeros((NC, 128, nt * 128), dtype=BF16)
    plan.slot_edge = np.full((NC, n_slots), -1, dtype=np.int64)

    eo = order
    core_s, win_s = core[eo], win[eo]
    grp = core_s * NWIN + win_s
    counts = np.bincount(grp, minlength=NC * NWIN)
    starts = np.concatenate([[0], np.cumsum(counts)[:-1]])
    pos = np.arange(len(eo)) - np.repeat(starts, counts)
    tile_in_win = pos // 128
    part = pos % 128
    tile_id = win_s * T_W + tile_in_win
    slot = tile_id * 128 + part

    plan.slot_edge[core_s, slot] = eo
    plan.msg_idx[core_s, part, tile_id] = (tr[src[eo]] * R + et[eo]).astype(np.int32)
    plan.u_idx[core_s, part, tile_id] = tr[src[eo]].astype(np.int32)
    m = (dst_local[eo] - win_s * 128)
    plan.S[core_s, part, tile_id * 128 + m] = s_e[eo].astype(BF16)
    plan.S0T[core_s, m, tile_id * 128 + part] = np.ones(len(eo), dtype=BF16)
    return plan


def _run_on_device(plan, in_maps):
    import concourse.bacc as bacc
    import concourse.mybir as mybir
    import concourse.tile as tile
    from concourse import bass_utils
    from concourse.bass import IndirectOffsetOnAxis

    T_W, NT = plan.T_W, plan.NT
    F = R * H  # 384
    bf = mybir.dt.bfloat16
    f32 = mybir.dt.float32
    Relu = mybir.ActivationFunctionType.Relu

    nc = bacc.Bacc("TRN2", target_bir_lowering=False, debug=False, num_devices=NC)

    # ---- I/O ----
    xT1_in = nc.dram_tensor("xT1", [IN, NPAD], bf, kind="ExternalInput")
    W1_in = nc.dram_tensor("W1", [128, 6 * 512], bf, kind="ExternalInput")
    W2_in = nc.dram_tensor("W2", [H, 512], bf, kind="ExternalInput")
    W3_in = nc.dram_tensor("W3", [H, 512], bf, kind="ExternalInput")
    Wuv_in = nc.dram_tensor("Wuv", [H, 260], bf, kind="ExternalInput")
    Wn1_in = nc.dram_tensor("Wn1", [H, 64], bf, kind="ExternalInput")
    Wn2_in = nc.dram_tensor("Wn2", [64, 2], bf, kind="ExternalInput")
    brow_in = nc.dram_tensor("brow", [1, 1536], bf, kind="ExternalInput")
    bcol_in = nc.dram_tensor("bcol", [128, 4], f32, kind="ExternalInput")
    # bcol columns: 0 = be1 (128), 1 = bn1 (64 used), 2 = be2 (3 used), 3 = bn2 (2 used)
    ident_in = nc.dram_tensor("ident", [128, 128], bf, kind="ExternalInput")
    midx_in = nc.dram_tensor("midx", [128, NT], mybir.dt.int32, kind="ExternalInput")
    uidx_in = nc.dram_tensor("uidx", [128, NT], mybir.dt.int32, kind="ExternalInput")
    S_in = nc.dram_tensor("S", [128, NT * 128], bf, kind="ExternalInput")
    S0T_in = nc.dram_tensor("S0T", [128, NT * 128], bf, kind="ExternalInput")

    eo_out = nc.dram_tensor("eo_out", [NWIN, 3, T_W * 128], f32, kind="ExternalOutput")
    no_out = nc.dram_tensor("no_out", [13, 2, 512], f32, kind="ExternalOutput")

    h_sh = [nc.dram_tensor(f"h_sh{l}", [NPAD, F], bf) for l in range(3)]
    tabs = [nc.dram_tensor(f"tab{l}", [N, F], bf, addr_space="Shared") for l in range(3)]
    tabvs = [t.ap().rearrange("n (r f) -> (n r) f", r=R) for t in tabs]
    u_sh = nc.dram_tensor("u_sh", [NPAD, H], bf)
    utab = nc.dram_tensor("utab", [N, H], bf, addr_space="Shared")

    def ag(nc_, inp, outp, rows, width):
        for c in range(NAG):
            nc_.gpsimd.collective_compute(
                "AllGather", mybir.AluOpType.bypass,
                replica_groups=[list(range(NC))],
                ins=[inp[c * CHUNK:(c + 1) * CHUNK, :]],
                outs=[outp[c * (NC * CHUNK):(c + 1) * (NC * CHUNK), :]],
            )

    with tile.TileContext(nc) as tc:
        with tc.tile_pool(name="const", bufs=1) as cp, \
             tc.tile_pool(name="sb", bufs=3) as sb, \
             tc.tile_pool(name="res", bufs=1) as rp:

            midx = cp.tile([128, NT], mybir.dt.int32)
            uidx = cp.tile([128, NT], mybir.dt.int32)
            nc.sync.dma_start(out=midx[:], in_=midx_in[:, :])
            nc.sync.dma_start(out=uidx[:], in_=uidx_in[:, :])
            W1 = cp.tile([128, 6 * 512], bf)
            nc.sync.dma_start(out=W1[:], in_=W1_in[:, :])
            W2 = cp.tile([128, 512], bf)
            W3 = cp.tile([128, 512], bf)
            Wuv = cp.tile([128, 256], bf)
            We2 = cp.tile([128, 3], bf)
            Wn1 = cp.tile([128, 64], bf)
            Wn2 = cp.tile([64, 2], bf)
            brow = cp.tile([1, 1536], bf)
            bcol = cp.tile([128, 4], f32)
            ident = cp.tile([128, 128], bf)
            nc.sync.dma_start(out=W2[:], in_=W2_in[:, :])
            nc.sync.dma_start(out=W3[:], in_=W3_in[:, :])
            nc.sync.dma_start(out=Wuv[:], in_=Wuv_in[:, 0:256])
            nc.sync.dma_start(out=We2[:], in_=Wuv_in[:, 256:259])
            nc.sync.dma_start(out=Wn1[:], in_=Wn1_in[:, :])
            nc.sync.dma_start(out=Wn2[:], in_=Wn2_in[:, :])
            nc.sync.dma_start(out=brow[:], in_=brow_in[:, :])
            nc.sync.dma_start(out=bcol[:], in_=bcol_in[:, :])
            nc.sync.dma_start(out=ident[:], in_=ident_in[:, :])
            ones1 = cp.tile([1, 128], bf)
            nc.any.memset(ones1[:], 1.0)

            root_sb = rp.tile([128, NPAD], f32)
            xT = []
            for l in range(3):
                xTl = rp.tile([128, NPAD], bf, tag=f"xT{l}", name=f"xT{l}")
                xT.append(xTl)
            v_sb = rp.tile([128, NPAD], bf)

            with tc.tile_pool(name="psA", bufs=2, space="PSUM") as psA, \
                 tc.tile_pool(name="psB", bufs=2, space="PSUM") as psB, \
                 tc.tile_pool(name="psC", bufs=2, space="PSUM") as psC:
                for l in range(3):
                    # ---- transform: h_sh[l] (384 cols) + root (128 cols) ----
                    for ntile in range(NWIN):
                        ps = psA.tile([128, 512], f32, tag="tf")
                        if l == 0:
                            for kt in range(6):
                                xt = sb.tile([128, 128], bf, tag="x1")
                                nc.sync.dma_start(
                                    out=xt[:],
                                    in_=xT1_in[kt * 128:(kt + 1) * 128,
                                               ntile * 128:(ntile + 1) * 128])
                                nc.tensor.matmul(out=ps[:], lhsT=xt[:],
                                                 rhs=W1[:, kt * 512:(kt + 1) * 512],
                                                 start=(kt == 0), stop=False)
                        else:
                            W = W2 if l == 1 else W3
                            nc.tensor.matmul(
                                out=ps[:], lhsT=xT[l - 1][:, ntile * 128:(ntile + 1) * 128],
                                rhs=W[:], start=True, stop=False)
                        nc.tensor.matmul(out=ps[:], lhsT=ones1[:],
                                         rhs=brow[0:1, l * 512:(l + 1) * 512], start=False, stop=True)
                        hsb = sb.tile([128, F], bf, tag="hsb")
                        nc.vector.tensor_copy(out=hsb[:], in_=ps[:, 0:F])
                        nc.sync.dma_start(
                            out=h_sh[l][ntile * 128:(ntile + 1) * 128, :], in_=hsb[:])
                        nc.vector.tensor_copy(
                            out=root_sb[:, ntile * 128:(ntile + 1) * 128],
                            in_=ps[:, F:512])
                    # ---- all-gather ----
                    ag(nc, h_sh[l], tabs[l], SHARD, F)
                    # ---- gather + scatter per window ----
                    for w in range(NWIN):
                        msg = sb.tile([128, T_W * 128], bf, tag="msg")
                        for t in range(T_W):
                            col = w * T_W + t
                            nc.gpsimd.indirect_dma_start(
                                out=msg[:, t * 128:(t + 1) * 128],
                                out_offset=None,
                                in_=tabvs[l],
                                in_offset=IndirectOffsetOnAxis(
                                    ap=midx[:, col:col + 1], axis=0),
                            )
                        Ssb = sb.tile([128, T_W * 128], bf, tag="S")
                        nc.sync.dma_start(
                            out=Ssb[:], in_=S_in[:, w * T_W * 128:(w + 1) * T_W * 128])
                        agg = psB.tile([128, H], f32, tag="agg")
                        for t in range(T_W):
                            nc.tensor.matmul(out=agg[:],
                                             lhsT=Ssb[:, t * 128:(t + 1) * 128],
                                             rhs=msg[:, t * 128:(t + 1) * 128],
                                             start=(t == 0), stop=(t == T_W - 1))
                        osum = sb.tile([128, H], f32, tag="osum")
                        nc.vector.tensor_add(
                            out=osum[:], in0=agg[:],
                            in1=root_sb[:, w * 128:(w + 1) * 128])
                        xb = sb.tile([128, H], bf, tag="xb")
                        if l < 2:
                            nc.scalar.activation(out=xb[:], in_=osum[:], func=Relu)
                        else:
                            nc.vector.tensor_copy(out=xb[:], in_=osum[:])
                        tp = psC.tile([128, H], f32, tag="tp")
                        nc.tensor.matmul(out=tp[:], lhsT=xb[:], rhs=ident[:],
                                         start=True, stop=True)
                        nc.vector.tensor_copy(
                            out=xT[l][:, w * 128:(w + 1) * 128], in_=tp[:])

                # ---- uv transform + node head (reuse pools) ----
                for ntile in range(NWIN):
                    ps = psA.tile([128, 256], f32, tag="tf")
                    nc.tensor.matmul(out=ps[:],
                                     lhsT=xT[2][:, ntile * 128:(ntile + 1) * 128],
                                     rhs=Wuv[:], start=True, stop=True)
                    usb = sb.tile([128, H], bf, tag="usb")
                    nc.vector.tensor_copy(out=usb[:], in_=ps[:, 0:H])
                    nc.sync.dma_start(
                        out=u_sh[ntile * 128:(ntile + 1) * 128, :], in_=usb[:])
                    nc.vector.tensor_copy(
                        out=v_sb[:, ntile * 128:(ntile + 1) * 128], in_=ps[:, H:256])
                ag(nc, u_sh, utab, SHARD, H)

                for seg in range(13):
                    c0 = seg * 512
                    cw = min(512, NPAD - c0)
                    h2 = psB.tile([64, 512], f32, tag="agg")
                    nc.tensor.matmul(out=h2[:, 0:cw], lhsT=Wn1[:],
                                     rhs=xT[2][:, c0:c0 + cw], start=True, stop=True)
                    h2s = sb.tile([64, 512], bf, tag="h2s")
                    nc.scalar.activation(out=h2s[:, 0:cw], in_=h2[:, 0:cw], func=Relu,
                                         bias=bcol[0:64, 1:2])
                    no = psC.tile([2, 512], f32, tag="tp")
                    nc.tensor.matmul(out=no[:, 0:cw], lhsT=Wn2[:],
                                     rhs=h2s[:, 0:cw], start=True, stop=True)
                    nos = sb.tile([2, 512], f32, tag="nos")
                    nc.vector.tensor_scalar_add(out=nos[:, 0:cw], in0=no[:, 0:cw],
                                                scalar1=bcol[0:2, 3:4])
                    nc.sync.dma_start(out=no_out[seg, :, 0:cw], in_=nos[:, 0:cw])

            # ---- edge head ----
            utabv = utab.ap()
            with tc.tile_pool(name="psH", bufs=1, space="PSUM") as psH, \
                 tc.tile_pool(name="psE", bufs=2, space="PSUM") as psE:
                for w in range(NWIN):
                    uex = sb.tile([128, T_W * 128], bf, tag="uex")
                    for t in range(T_W):
                        col = w * T_W + t
                        nc.gpsimd.indirect_dma_start(
                            out=uex[:, t * 128:(t + 1) * 128],
                            out_offset=None,
                            in_=utabv,
                            in_offset=IndirectOffsetOnAxis(
                                ap=uidx[:, col:col + 1], axis=0),
                        )
                    S0s = sb.tile([128, T_W * 128], bf, tag="S0")
                    nc.sync.dma_start(
                        out=S0s[:], in_=S0T_in[:, w * T_W * 128:(w + 1) * T_W * 128])
                    hT = psH.tile([128, T_W * 128], f32, tag="hT")
                    for t in range(T_W):
                        sl = slice(t * 128, (t + 1) * 128)
                        nc.tensor.matmul(out=hT[:, sl], lhsT=uex[:, sl],
                                         rhs=ident[:], start=True, stop=False)
                        nc.tensor.matmul(out=hT[:, sl],
                                         lhsT=v_sb[:, w * 128:(w + 1) * 128],
                                         rhs=S0s[:, sl], start=False, stop=True)
                    hTs = sb.tile([128, T_W * 128], bf, tag="hTs")
                    nc.scalar.activation(out=hTs[:], in_=hT[:], func=Relu,
                                         bias=bcol[:, 0:1])
                    c0 = 0
                    while c0 < T_W * 128:
                        cw = min(512, T_W * 128 - c0)
                        eo = psE.tile([3, 512], f32, tag="eo")
                        nc.tensor.matmul(out=eo[:, 0:cw], lhsT=We2[:],
                                         rhs=hTs[:, c0:c0 + cw], start=True, stop=True)
                        eos = sb.tile([3, 512], f32, tag="eos")
                        nc.vector.tensor_scalar_add(out=eos[:, 0:cw], in0=eo[:, 0:cw],
                                                    scalar1=bcol[0:3, 2:3])
                        nc.sync.dma_start(out=eo_out[w, :, c0:c0 + cw],
                                          in_=eos[:, 0:cw])
                        c0 += cw

    nc.compile()
    import os
    trace = os.environ.get("KTRACE") == "1"
    res = bass_utils.run_bass_kernel_spmd(nc, in_maps, core_ids=list(range(NC)),
                                          trace=trace)
    global LAST_EXEC_NS
    LAST_EXEC_NS = res.exec_time_ns
    return res


def kernel(x, edge_index, edge_type,
           W_rel1, W_root1, b1, W_rel2, W_root2, b2, W_rel3, W_root3, b3,
           We1, be1, We2, be2, Wn1, bn1, Wn2, bn2):
    x = np.asarray(x, dtype=np.float32)
    ei = np.asarray(edge_index)
    et = np.asarray(edge_type).astype(np.int64)
    src, dst = ei[0].astype(np.int64), ei[1].astype(np.int64)

    plan = _build_plan(src, dst, et)

    W1cat = np.concatenate([np.asarray(W_rel1[r], np.float32) for r in range(R)]
                           + [np.asarray(W_root1, np.float32)], axis=1).astype(BF16)
    W1cat = W1cat.reshape(6, 128, 512).transpose(1, 0, 2).reshape(128, 6 * 512)
    W2cat = np.concatenate([np.asarray(W_rel2[r], np.float32) for r in range(R)]
                           + [np.asarray(W_root2, np.float32)], axis=1).astype(BF16)
    W3cat = np.concatenate([np.asarray(W_rel3[r], np.float32) for r in range(R)]
                           + [np.asarray(W_root3, np.float32)], axis=1).astype(BF16)
    We1a = np.asarray(We1, np.float32)
    Wuv = np.concatenate([We1a[:H, :], We1a[H:, :]], axis=1).astype(BF16)  # [128, 256]
    brow = np.zeros((1, 1536), dtype=BF16)
    for i, b in enumerate([b1, b2, b3]):
        brow[0, i * 512 + R * H:(i + 1) * 512] = np.asarray(b, np.float32).astype(BF16)
    bcol = np.zeros((128, 4), dtype=np.float32)
    bcol[:, 0] = np.asarray(be1, np.float32)
    bcol[:64, 1] = np.asarray(bn1, np.float32)
    bcol[:3, 2] = np.asarray(be2, np.float32)
    bcol[:2, 3] = np.asarray(bn2, np.float32)
    Wuv_ext = np.zeros((H, 260), dtype=BF16)
    Wuv_ext[:, :256] = Wuv
    Wuv_ext[:, 256:259] = np.asarray(We2, np.float32).astype(BF16)

    ident = np.eye(128, dtype=BF16)

    in_maps = []
    for c in range(NC):
        xs = x[c * SHARD:(c + 1) * SHARD, :]       # [6250, 768]
        xT = np.zeros((IN, NPAD), dtype=BF16)
        xT[:, :SHARD] = xs.T.astype(BF16)
        in_maps.append({
            "xT1": xT,
            "W1": W1cat, "W2": W2cat, "W3": W3cat,
            "Wuv": Wuv_ext,
            "Wn1": np.asarray(Wn1, np.float32).astype(BF16),
            "Wn2": np.asarray(Wn2, np.float32).astype(BF16),
            "brow": brow, "bcol": bcol, "ident": ident,
            "midx": plan.msg_idx[c], "uidx": plan.u_idx[c],
            "S": plan.S[c], "S0T": plan.S0T[c],
        })

    res = _run_on_device(plan, in_maps)

    T_W = plan.T_W
    edge_out = np.zeros((E, 3), dtype=np.float32)
    node_out = np.zeros((N, 2), dtype=np.float32)
    for c in range(NC):
        eo = res.results[c]["eo_out"]          # [NWIN, 3, T_W*128]
        no = res.results[c]["no_out"]          # [13, 2, 512]
        se = plan.slot_edge[c]                 # [n_slots]
        valid = se >= 0
        slots = np.nonzero(valid)[0]
        w = slots // (T_W * 128)
        s = slots % (T_W * 128)
        edge_out[se[slots], :] = eo[w, :, s]
        nvals = no.transpose(0, 2, 1).reshape(13 * 512, 2)[:SHARD, :]
        node_out[c * SHARD:(c + 1) * SHARD, :] = nvals
    return edge_out, node_out


# revision 6
# speedup vs baseline: 1.0658x; 1.0658x over previous
"""Trainium2 Bass kernel for EnhancedLegalRGCN (3-layer RGCN + edge/node heads).

Strategy (8 NeuronCores, SPMD):
- Nodes block-sharded 8 ways (6250/core). Edges owned by dst core, packed into
  dst-window-aligned slots (49 windows x T_W tiles x 128 edges, padded).
- Per layer: sharded transform GEMM (x^T tiles @ [W_rel|W_root]) -> AllGather
  bf16 table [50000, 384] in 5 chunks -> per-edge indirect DMA gather of
  (src, rel) message rows -> scatter via host-built one-hot*1/cnt S matrices
  with TensorE matmuls accumulating each dst window in PSUM -> +root, ReLU.
- Edge head: u = x3@We1[:H] gathered per edge; v = x3@We1[H:] expanded from the
  local dst window by a one-hot matmul; hidden^T built directly in PSUM
  (transpose-matmul + v-expansion matmul), ReLU+bias on ScalarE, @We2.
- Node head: local GEMMs in transposed space.
Host reassembles/unpermutes outputs.
"""
import numpy as np
import ml_dtypes

BF16 = ml_dtypes.bfloat16
LAST_EXEC_NS = None

N, E, IN, H, R = 50000, 800000, 768, 128, 3
NC = 8
SHARD = N // NC          # 6250
NWIN = 49                # ceil(6250/128)
NAG = 5
CHUNK = SHARD // NAG     # 1250
NPAD = NWIN * 128        # 6272


def _table_row(n):
    k = n // SHARD
    m = n % SHARD
    c = m // CHUNK
    p = m % CHUNK
    return c * (NC * CHUNK) + k * CHUNK + p


def _build_plan(src, dst, et):
    cnt = np.zeros((R, N), dtype=np.int64)
    np.add.at(cnt, (et, dst), 1)
    scale_rn = 1.0 / np.maximum(cnt, 1).astype(np.float32)
    s_e = scale_rn[et, dst].astype(np.float32)

    core = dst // SHARD
    dst_local = dst % SHARD
    win = dst_local // 128

    order = np.lexsort((win, core))
    cw_cnt = np.zeros((NC, NWIN), dtype=np.int64)
    np.add.at(cw_cnt, (core, win), 1)
    T_W = int(np.ceil(cw_cnt.max() / 128))
    nt = NWIN * T_W
    n_slots = nt * 128

    tr = _table_row(np.arange(N))

    class P:
        pass
    plan = P()
    plan.T_W = T_W
    plan.NT = nt
    plan.msg_idx = np.zeros((NC, 128, nt), dtype=np.int32)
    plan.u_idx = np.zeros((NC, 128, nt), dtype=np.int32)
    plan.S = np.zeros((NC, 128, nt * 128), dtype=BF16)
    plan.S0T = np.zeros((NC, 128, nt * 128), dtype=BF16)
    plan.slot_edge = np.full((NC, n_slots), -1, dtype=np.int64)

    eo = order
    core_s, win_s = core[eo], win[eo]
    grp = core_s * NWIN + win_s
    counts = np.bincount(grp, minlength=NC * NWIN)
    starts = np.concatenate([[0], np.cumsum(counts)[:-1]])
    pos = np.arange(len(eo)) - np.repeat(starts, counts)
    tile_in_win = pos // 128
    part = pos % 128
    tile_id = win_s * T_W + tile_in_win
    slot = tile_id * 128 + part

    plan.slot_edge[core_s, slot] = eo
    plan.msg_idx[core_s, part, tile_id] = (tr[src[eo]] * R + et[eo]).astype(np.int32)
    plan.u_idx[core_s, part, tile_id] = tr[src[eo]].astype(np.int32)
    m = (dst_local[eo] - win_s * 128)
    plan.S[core_s, part, tile_id * 128 + m] = s_e[eo].astype(BF16)
    plan.S0T[core_s, m, tile_id * 128 + part] = np.ones(len(eo), dtype=BF16)
    return plan


def _run_on_device(plan, in_maps):
    import concourse.bacc as bacc
    import concourse.mybir as mybir
    import concourse.tile as tile
    from concourse import bass_utils
    from concourse.bass import IndirectOffsetOnAxis

    T_W, NT = plan.T_W, plan.NT
    F = R * H  # 384
    bf = mybir.dt.bfloat16
    f32 = mybir.dt.float32
    Relu = mybir.ActivationFunctionType.Relu

    nc = bacc.Bacc("TRN2", target_bir_lowering=False, debug=False, num_devices=NC)

    # ---- I/O ----
    xT1_in = nc.dram_tensor("xT1", [IN, NPAD], bf, kind="ExternalInput")
    W1_in = nc.dram_tensor("W1", [128, 6 * 512], bf, kind="ExternalInput")
    W2_in = nc.dram_tensor("W2", [H, 512], bf, kind="ExternalInput")
    W3_in = nc.dram_tensor("W3", [H, 512], bf, kind="ExternalInput")
    Wuv_in = nc.dram_tensor("Wuv", [H, 260], bf, kind="ExternalInput")
    Wn1_in = nc.dram_tensor("Wn1", [H, 64], bf, kind="ExternalInput")
    Wn2_in = nc.dram_tensor("Wn2", [64, 2], bf, kind="ExternalInput")
    brow_in = nc.dram_tensor("brow", [1, 1536], bf, kind="ExternalInput")
    bcol_in = nc.dram_tensor("bcol", [128, 4], f32, kind="ExternalInput")
    # bcol columns: 0 = be1 (128), 1 = bn1 (64 used), 2 = be2 (3 used), 3 = bn2 (2 used)
    ident_in = nc.dram_tensor("ident", [128, 128], bf, kind="ExternalInput")
    midx_in = nc.dram_tensor("midx", [128, NT], mybir.dt.int32, kind="ExternalInput")
    uidx_in = nc.dram_tensor("uidx", [128, NT], mybir.dt.int32, kind="ExternalInput")
    S_in = nc.dram_tensor("S", [128, NT * 128], bf, kind="ExternalInput")
    S0T_in = nc.dram_tensor("S0T", [128, NT * 128], bf, kind="ExternalInput")

    eo_out = nc.dram_tensor("eo_out", [NWIN, 3, T_W * 128], f32, kind="ExternalOutput")
    no_out = nc.dram_tensor("no_out", [13, 2, 512], f32, kind="ExternalOutput")

    h_sh = [nc.dram_tensor(f"h_sh{l}", [NPAD, F], bf) for l in range(3)]
    tabs = [nc.dram_tensor(f"tab{l}", [N, F], bf, addr_space="Shared") for l in range(3)]
    tabvs = [t.ap().rearrange("n (r f) -> (n r) f", r=R) for t in tabs]
    u_sh = nc.dram_tensor("u_sh", [NPAD, H], bf)
    utab = nc.dram_tensor("utab", [N, H], bf, addr_space="Shared")

    with tile.TileContext(nc) as tc:
        with tc.tile_pool(name="const", bufs=1) as cp, \
             tc.tile_pool(name="sb", bufs=3) as sb, \
             tc.tile_pool(name="res", bufs=1) as rp:

            midx = cp.tile([128, NT], mybir.dt.int32)
            uidx = cp.tile([128, NT], mybir.dt.int32)
            nc.sync.dma_start(out=midx[:], in_=midx_in[:, :])
            nc.sync.dma_start(out=uidx[:], in_=uidx_in[:, :])
            W1 = cp.tile([128, 6 * 512], bf)
            nc.sync.dma_start(out=W1[:], in_=W1_in[:, :])
            W2 = cp.tile([128, 512], bf)
            W3 = cp.tile([128, 512], bf)
            Wuv = cp.tile([128, 256], bf)
            We2 = cp.tile([128, 3], bf)
            Wn1 = cp.tile([128, 64], bf)
            Wn2 = cp.tile([64, 2], bf)
            brow = cp.tile([1, 1536], bf)
            bcol = cp.tile([128, 4], f32)
            ident = cp.tile([128, 128], bf)
            nc.sync.dma_start(out=W2[:], in_=W2_in[:, :])
            nc.sync.dma_start(out=W3[:], in_=W3_in[:, :])
            nc.sync.dma_start(out=Wuv[:], in_=Wuv_in[:, 0:256])
            nc.sync.dma_start(out=We2[:], in_=Wuv_in[:, 256:259])
            nc.sync.dma_start(out=Wn1[:], in_=Wn1_in[:, :])
            nc.sync.dma_start(out=Wn2[:], in_=Wn2_in[:, :])
            nc.sync.dma_start(out=brow[:], in_=brow_in[:, :])
            nc.sync.dma_start(out=bcol[:], in_=bcol_in[:, :])
            nc.sync.dma_start(out=ident[:], in_=ident_in[:, :])
            ones1 = cp.tile([1, 128], bf)
            nc.any.memset(ones1[:], 1.0)

            root_sb = rp.tile([128, NPAD], f32)
            xT = []
            for l in range(3):
                xTl = rp.tile([128, NPAD], bf, tag=f"xT{l}", name=f"xT{l}")
                xT.append(xTl)
            v_sb = rp.tile([128, NPAD], bf)

            import math
            AG_THR = [int(math.ceil(CHUNK * (c + 1) / 128)) for c in range(NAG)]

            with tc.tile_pool(name="psA", bufs=2, space="PSUM") as psA, \
                 tc.tile_pool(name="psB", bufs=2, space="PSUM") as psB, \
                 tc.tile_pool(name="psC", bufs=2, space="PSUM") as psC:

                def ag_chunk(dst_l, c):
                    inp = h_sh[dst_l] if dst_l < 3 else u_sh
                    outp = tabs[dst_l] if dst_l < 3 else utab
                    nc.gpsimd.collective_compute(
                        "AllGather", mybir.AluOpType.bypass,
                        replica_groups=[list(range(NC))],
                        ins=[inp[c * CHUNK:(c + 1) * CHUNK, :]],
                        outs=[outp[c * (NC * CHUNK):(c + 1) * (NC * CHUNK), :]],
                    )

                def transform_tile(l, ntile):
                    # layer l in 0..2: h_sh[l] (384 cols) + root (128 cols)
                    # l == 3: uv transform -> u_sh + v_sb
                    if l == 3:
                        ps = psA.tile([128, 256], f32, tag="tf", name="ps_uv")
                        nc.tensor.matmul(out=ps[:],
                                         lhsT=xT[2][:, ntile * 128:(ntile + 1) * 128],
                                         rhs=Wuv[:], start=True, stop=True)
                        usb = sb.tile([128, H], bf, tag="usb", name="usb")
                        nc.vector.tensor_copy(out=usb[:], in_=ps[:, 0:H])
                        nc.sync.dma_start(
                            out=u_sh[ntile * 128:(ntile + 1) * 128, :], in_=usb[:])
                        nc.vector.tensor_copy(
                            out=v_sb[:, ntile * 128:(ntile + 1) * 128], in_=ps[:, H:256])
                        return
                    ps = psA.tile([128, 512], f32, tag="tf", name="ps_tf")
                    if l == 0:
                        for kt in range(6):
                            xt = sb.tile([128, 128], bf, tag="x1", name="xt")
                            nc.sync.dma_start(
                                out=xt[:],
                                in_=xT1_in[kt * 128:(kt + 1) * 128,
                                           ntile * 128:(ntile + 1) * 128])
                            nc.tensor.matmul(out=ps[:], lhsT=xt[:],
                                             rhs=W1[:, kt * 512:(kt + 1) * 512],
                                             start=(kt == 0), stop=False)
                    else:
                        W = W2 if l == 1 else W3
                        nc.tensor.matmul(
                            out=ps[:], lhsT=xT[l - 1][:, ntile * 128:(ntile + 1) * 128],
                            rhs=W[:], start=True, stop=False)
                    nc.tensor.matmul(out=ps[:], lhsT=ones1[:],
                                     rhs=brow[0:1, l * 512:(l + 1) * 512],
                                     start=False, stop=True)
                    hsb = sb.tile([128, F], bf, tag="hsb", name="hsb")
                    nc.vector.tensor_copy(out=hsb[:], in_=ps[:, 0:F])
                    nc.sync.dma_start(
                        out=h_sh[l][ntile * 128:(ntile + 1) * 128, :], in_=hsb[:])
                    nc.vector.tensor_copy(
                        out=root_sb[:, ntile * 128:(ntile + 1) * 128],
                        in_=ps[:, F:512])

                # layer-0 transform, AG chunks issued as rows complete
                for ntl in range(NWIN):
                    transform_tile(0, ntl)
                    for c in range(NAG):
                        if AG_THR[c] == ntl + 1:
                            ag_chunk(0, c)

                for l in range(3):
                    for w in range(NWIN):
                        msg = sb.tile([128, T_W * 128], bf, tag="msg", name="msg")
                        for t in range(T_W):
                            col = w * T_W + t
                            nc.gpsimd.indirect_dma_start(
                                out=msg[:, t * 128:(t + 1) * 128],
                                out_offset=None,
                                in_=tabvs[l],
                                in_offset=IndirectOffsetOnAxis(
                                    ap=midx[:, col:col + 1], axis=0),
                            )
                        Ssb = sb.tile([128, T_W * 128], bf, tag="S", name="Ssb")
                        nc.sync.dma_start(
                            out=Ssb[:], in_=S_in[:, w * T_W * 128:(w + 1) * T_W * 128])
                        agg = psB.tile([128, H], f32, tag="agg", name="agg")
                        for t in range(T_W):
                            nc.tensor.matmul(out=agg[:],
                                             lhsT=Ssb[:, t * 128:(t + 1) * 128],
                                             rhs=msg[:, t * 128:(t + 1) * 128],
                                             start=(t == 0), stop=(t == T_W - 1))
                        osum = sb.tile([128, H], f32, tag="osum", name="osum")
                        nc.vector.tensor_add(
                            out=osum[:], in0=agg[:],
                            in1=root_sb[:, w * 128:(w + 1) * 128])
                        xb = sb.tile([128, H], bf, tag="xb", name="xb")
                        if l < 2:
                            nc.scalar.activation(out=xb[:], in_=osum[:], func=Relu)
                        else:
                            nc.vector.tensor_copy(out=xb[:], in_=osum[:])
                        tp = psC.tile([128, H], f32, tag="tp", name="tp")
                        nc.tensor.matmul(out=tp[:], lhsT=xb[:], rhs=ident[:],
                                         start=True, stop=True)
                        nc.vector.tensor_copy(
                            out=xT[l][:, w * 128:(w + 1) * 128], in_=tp[:])
                        # pipeline: transform for the next stage + its AG chunks
                        transform_tile(l + 1, w)
                        for c in range(NAG):
                            if AG_THR[c] == w + 1:
                                ag_chunk(l + 1, c)

                for seg in range(13):
                    c0 = seg * 512
                    cw = min(512, NPAD - c0)
                    h2 = psB.tile([64, 512], f32, tag="agg", name="h2")
                    nc.tensor.matmul(out=h2[:, 0:cw], lhsT=Wn1[:],
                                     rhs=xT[2][:, c0:c0 + cw], start=True, stop=True)
                    h2s = sb.tile([64, 512], bf, tag="h2s", name="h2s")
                    nc.scalar.activation(out=h2s[:, 0:cw], in_=h2[:, 0:cw], func=Relu,
                                         bias=bcol[0:64, 1:2])
                    no = psC.tile([2, 512], f32, tag="tp", name="no")
                    nc.tensor.matmul(out=no[:, 0:cw], lhsT=Wn2[:],
                                     rhs=h2s[:, 0:cw], start=True, stop=True)
                    nos = sb.tile([2, 512], f32, tag="nos", name="nos")
                    nc.vector.tensor_scalar_add(out=nos[:, 0:cw], in0=no[:, 0:cw],
                                                scalar1=bcol[0:2, 3:4])
                    nc.sync.dma_start(out=no_out[seg, :, 0:cw], in_=nos[:, 0:cw])

            # ---- edge head ----
            utabv = utab.ap()
            with tc.tile_pool(name="psH", bufs=1, space="PSUM") as psH, \
                 tc.tile_pool(name="psE", bufs=2, space="PSUM") as psE:
                for w in range(NWIN):
                    uex = sb.tile([128, T_W * 128], bf, tag="uex")
                    for t in range(T_W):
                        col = w * T_W + t
                        nc.gpsimd.indirect_dma_start(
                            out=uex[:, t * 128:(t + 1) * 128],
                            out_offset=None,
                            in_=utabv,
                            in_offset=IndirectOffsetOnAxis(
                                ap=uidx[:, col:col + 1], axis=0),
                        )
                    S0s = sb.tile([128, T_W * 128], bf, tag="S0")
                    nc.sync.dma_start(
                        out=S0s[:], in_=S0T_in[:, w * T_W * 128:(w + 1) * T_W * 128])
                    hT = psH.tile([128, T_W * 128], f32, tag="hT")
                    for t in range(T_W):
                        sl = slice(t * 128, (t + 1) * 128)
                        nc.tensor.matmul(out=hT[:, sl], lhsT=uex[:, sl],
                                         rhs=ident[:], start=True, stop=False)
                        nc.tensor.matmul(out=hT[:, sl],
                                         lhsT=v_sb[:, w * 128:(w + 1) * 128],
                                         rhs=S0s[:, sl], start=False, stop=True)
                    hTs = sb.tile([128, T_W * 128], bf, tag="hTs")
                    nc.scalar.activation(out=hTs[:], in_=hT[:], func=Relu,
                                         bias=bcol[:, 0:1])
                    c0 = 0
                    while c0 < T_W * 128:
                        cw = min(512, T_W * 128 - c0)
                        eo = psE.tile([3, 512], f32, tag="eo")
                        nc.tensor.matmul(out=eo[:, 0:cw], lhsT=We2[:],
                                         rhs=hTs[:, c0:c0 + cw], start=True, stop=True)
                        eos = sb.tile([3, 512], f32, tag="eos")
                        nc.vector.tensor_scalar_add(out=eos[:, 0:cw], in0=eo[:, 0:cw],
                                                    scalar1=bcol[0:3, 2:3])
                        nc.sync.dma_start(out=eo_out[w, :, c0:c0 + cw],
                                          in_=eos[:, 0:cw])
                        c0 += cw

    nc.compile()
    import os
    trace = os.environ.get("KTRACE") == "1"
    res = bass_utils.run_bass_kernel_spmd(nc, in_maps, core_ids=list(range(NC)),
                                          trace=trace)
    global LAST_EXEC_NS
    LAST_EXEC_NS = res.exec_time_ns
    return res


def kernel(x, edge_index, edge_type,
           W_rel1, W_root1, b1, W_rel2, W_root2, b2, W_rel3, W_root3, b3,
           We1, be1, We2, be2, Wn1, bn1, Wn2, bn2):
    x = np.asarray(x, dtype=np.float32)
    ei = np.asarray(edge_index)
    et = np.asarray(edge_type).astype(np.int64)
    src, dst = ei[0].astype(np.int64), ei[1].astype(np.int64)

    plan = _build_plan(src, dst, et)

    W1cat = np.concatenate([np.asarray(W_rel1[r], np.float32) for r in range(R)]
                           + [np.asarray(W_root1, np.float32)], axis=1).astype(BF16)
    W1cat = W1cat.reshape(6, 128, 512).transpose(1, 0, 2).reshape(128, 6 * 512)
    W2cat = np.concatenate([np.asarray(W_rel2[r], np.float32) for r in range(R)]
                           + [np.asarray(W_root2, np.float32)], axis=1).astype(BF16)
    W3cat = np.concatenate([np.asarray(W_rel3[r], np.float32) for r in range(R)]
                           + [np.asarray(W_root3, np.float32)], axis=1).astype(BF16)
    We1a = np.asarray(We1, np.float32)
    Wuv = np.concatenate([We1a[:H, :], We1a[H:, :]], axis=1).astype(BF16)  # [128, 256]
    brow = np.zeros((1, 1536), dtype=BF16)
    for i, b in enumerate([b1, b2, b3]):
        brow[0, i * 512 + R * H:(i + 1) * 512] = np.asarray(b, np.float32).astype(BF16)
    bcol = np.zeros((128, 4), dtype=np.float32)
    bcol[:, 0] = np.asarray(be1, np.float32)
    bcol[:64, 1] = np.asarray(bn1, np.float32)
    bcol[:3, 2] = np.asarray(be2, np.float32)
    bcol[:2, 3] = np.asarray(bn2, np.float32)
    Wuv_ext = np.zeros((H, 260), dtype=BF16)
    Wuv_ext[:, :256] = Wuv
    Wuv_ext[:, 256:259] = np.asarray(We2, np.float32).astype(BF16)

    ident = np.eye(128, dtype=BF16)

    in_maps = []
    for c in range(NC):
        xs = x[c * SHARD:(c + 1) * SHARD, :]       # [6250, 768]
        xT = np.zeros((IN, NPAD), dtype=BF16)
        xT[:, :SHARD] = xs.T.astype(BF16)
        in_maps.append({
            "xT1": xT,
            "W1": W1cat, "W2": W2cat, "W3": W3cat,
            "Wuv": Wuv_ext,
            "Wn1": np.asarray(Wn1, np.float32).astype(BF16),
            "Wn2": np.asarray(Wn2, np.float32).astype(BF16),
            "brow": brow, "bcol": bcol, "ident": ident,
            "midx": plan.msg_idx[c], "uidx": plan.u_idx[c],
            "S": plan.S[c], "S0T": plan.S0T[c],
        })

    res = _run_on_device(plan, in_maps)

    T_W = plan.T_W
    edge_out = np.zeros((E, 3), dtype=np.float32)
    node_out = np.zeros((N, 2), dtype=np.float32)
    for c in range(NC):
        eo = res.results[c]["eo_out"]          # [NWIN, 3, T_W*128]
        no = res.results[c]["no_out"]          # [13, 2, 512]
        se = plan.slot_edge[c]                 # [n_slots]
        valid = se >= 0
        slots = np.nonzero(valid)[0]
        w = slots // (T_W * 128)
        s = slots % (T_W * 128)
        edge_out[se[slots], :] = eo[w, :, s]
        nvals = no.transpose(0, 2, 1).reshape(13 * 512, 2)[:SHARD, :]
        node_out[c * SHARD:(c + 1) * SHARD, :] = nvals
    return edge_out, node_out
